# revision 31
# baseline (speedup 1.0000x reference)
"""Trainium2 Bass kernel for nn_Attention_77824807403911 (sparse_attention).

Math (per batch element, no softmax => associativity):
    q = x @ Wq^T + bq ; v = x @ Wv^T + bv          [1024, 256]
    rq = rope(q) ; rv = rope(v)
    per head h (16 heads, hd=16):  att_h = rq_h @ (rq_h^T @ rv_h) / 4
    out = att @ Wo^T + bo

Instead of the 1024x1024 score matrix we compute the 16x16 Gram per head
(64x fewer flops), realized as a full 256x256 Gram masked to the
block-diagonal, folded with Wo into a single per-batch [256,256] weight:
    F[e,f]  = sum_s rv[s,e] rq[s,f]       (Gram, transposed blocks)
    BDT     = F .* (blockmask/4)
    W2[f,o] = sum_e BDT[e,f] * Wo[o,e]
    outT    = W2^T @ rqT + bo             ([256, 1024])

Sharding: data-parallel over batch, 1 element per core, no collectives.

v12 deltas over v6:
- trig tables are no longer DMAed as [256,1024] (512KB): only 2 unique
  rows exist per table (theta in {1, 1e-4}), so a [4,1152] aux tensor
  (rows + 0/1 indicator columns) is row-broadcast on the PE via two-
  partition indicator matmuls, saving ~500KB of input traffic;
- mask carries the 1/sqrt(hd)=1/4 scale (host-folded), W2 evict is a
  plain copy;
- three DMA queues: HW rings carry wq|wv + most of x; the Pool SWDGE
  queue (4KB packets, otherwise-idle engine) carries the trig rows, one
  x piece and the late-needed wo|mask block;
- rope stays DVE-only and evicts Scalar-only: concurrent GpSimd
  elementwise slows DVE ~3x (shared SBUF read ports), while Scalar's
  PSUM-side ACTIVATE does not contend;
- rope op order is E-products first (frees each PSUM bank early) and q
  before v; per-piece half-width transposes fire right after each
  real/imag combine so the gram's first s-chunks accumulate during rope;
- PSUM: warm+trig share a 3-buffer tag, matmuls rotate through 5.
"""

import numpy as np
import ml_dtypes

import concourse.bass as bass
import concourse.bacc as bacc
import concourse.tile as tile
from concourse import mybir
from concourse.bass_utils import run_bass_kernel_spmd

B, S, D, H, HD = 8, 1024, 256, 16, 16
N_CORES = 8
BF16 = mybir.dt.bfloat16
F32 = mybir.dt.float32

PERM = np.concatenate(
    [np.arange(0, 128, 2), np.arange(128, 256, 2),
     np.arange(1, 128, 2), np.arange(129, 256, 2)]
)

WB = 4 * D + 4          # 1028 packed weight columns
A_END = 2 * D + 4       # wq|wv|bias3|pad piece


def _host_tables():
    s = np.arange(S, dtype=np.float64) + 1.0
    trig = np.stack([np.sin(s), np.sin(1e-4 * s),
                     np.cos(s), np.cos(1e-4 * s)])
    # indicator columns: row-broadcast weights (partition p < 64 -> row 0)
    p = np.arange(128)
    ind = np.stack([(p < 64), (p >= 64), (p < 64), (p >= 64)])
    aux = np.concatenate([trig, ind], axis=1).astype(ml_dtypes.bfloat16)
    a = np.arange(256)
    headp = (a % 128) // 8
    mask = 0.25 * (headp[:, None] == headp[None, :])
    return aux, mask.astype(ml_dtypes.bfloat16)


def build_kernel():
    nc = bacc.Bacc()
    xT = nc.declare_dram_parameter("xT", [D, S], BF16, isOutput=False)
    # wbig columns: [wq(256) | wv(256) | bias(3) | pad(1) | wo(256) | mask(256)]
    wbig = nc.declare_dram_parameter("wbig", [D, WB], BF16, isOutput=False)
    # aux rows: sinA, sinB, cosA, cosB (theta=1, 1e-4) + indicator cols
    aux = nc.declare_dram_parameter("aux", [4, S + 128], BF16, isOutput=False)
    outT = nc.declare_dram_parameter("outT", [D, S], BF16, isOutput=True)

    with tile.TileContext(nc) as tc:
        _body(tc, xT, wbig, aux, outT)
    nc.compile()
    return nc


def _body(tc, xT, wbig, aux, outT):
    nc = tc.nc
    NS = 2          # s chunks of 512 for matmul streaming
    SC = S // NS    # 512

    with (
        tc.tile_pool(name="const", bufs=1) as cpool,
        tc.tile_pool(name="acts", bufs=1) as apool,
        tc.tile_pool(name="psum", bufs=4, space="PSUM") as pp,
        tc.tile_pool(name="outp", bufs=4) as opool,
    ):
        # PE warm-up: garbage matmuls release the HAM clock gate while the
        # real inputs stream in.
        scratch = cpool.tile([128, 512], BF16, tag="scratch", name="scratch")
        nc.gpsimd.memset(scratch[:], 0.25)
        warm_ps = pp.tile([128, 512], F32, tag="warm", bufs=3, name="warm_ps")
        for wi in range(4):
            nc.tensor.matmul(warm_ps[:], scratch[:, 0:128], scratch[:],
                             start=True, stop=True, skip_group_check=True)

        xT_sb, w_sb = [], []
        for cc in range(2):
            xT_sb.append(cpool.tile([128, S], BF16, tag=f"xT{cc}", name=f"xT{cc}"))
            w_sb.append(cpool.tile([128, WB], BF16, tag=f"wbig{cc}",
                                   name=f"wbig{cc}"))
        sin_sb = cpool.tile([128, S], BF16, tag="sin", name="sin_sb")
        cos_sb = cpool.tile([128, S], BF16, tag="cos", name="cos_sb")
        sinrows = cpool.tile([2, S + 128], BF16, tag="sinrows", name="sinrows")
        cosrows = cpool.tile([2, S + 128], BF16, tag="cosrows", name="cosrows")

        # ---- input DMAs: HW rings carry only wq|wv + x (the critical
        # path); tiny aux rows + late-needed wo|mask ride the Pool SWDGE
        # queue (4KB packets, otherwise-idle engine) ----
        # three queues: HW rings carry wq + most of x; the Pool SWDGE
        # queue carries the trig rows, one x piece, and the late wo|mask
        nc.gpsimd.dma_start(sinrows[:], aux[0:2, :])
        nc.gpsimd.dma_start(cosrows[:], aux[2:4, :])
        nc.sync.dma_start(w_sb[0][:, 0:D], wbig[0:128, 0:D])
        nc.scalar.dma_start(w_sb[1][:, 0:D], wbig[128:256, 0:D])
        nc.sync.dma_start(xT_sb[0][:, 0:SC], xT[0:128, 0:SC])
        nc.scalar.dma_start(xT_sb[1][:, 0:SC], xT[128:256, 0:SC])
        nc.gpsimd.dma_start(xT_sb[1][:, SC:S], xT[128:256, SC:S])
        nc.sync.dma_start(xT_sb[0][:, SC:S], xT[0:128, SC:S])
        nc.scalar.dma_start(w_sb[1][:, D:A_END], wbig[128:256, D:A_END])
        nc.sync.dma_start(w_sb[0][:, D:A_END], wbig[0:128, D:A_END])
        nc.gpsimd.dma_start(w_sb[0][:, A_END:WB], wbig[0:128, A_END:WB])
        nc.gpsimd.dma_start(w_sb[1][:, A_END:WB], wbig[128:256, A_END:WB])

        # ---- trig tables: PE row-broadcast (only 2 unique rows/table);
        # pinned to the front of the PE stream so the monotonic engine
        # counters don't chain them behind x-gated projection matmuls ----
        trig_ps = {}
        with tc.high_priority():
            # s0 halves of both tables first (earliest rope consumers)
            for t, half, rows in ((0, 0, sinrows), (1, 0, cosrows),
                                  (0, 1, sinrows), (1, 1, cosrows)):
                ps = pp.tile([128, SC], F32, tag="warm", bufs=3,
                             name=f"trig_ps{t}{half}")
                nc.tensor.matmul(
                    ps[:],
                    rows[0:2, S:S + 128],
                    rows[0:2, half * SC:(half + 1) * SC],
                    start=True, stop=True,
                )
                trig_ps[(t, half)] = ps
            # sin evicts in the DVE bubble, cos evicts in the Scalar bubble
            for half in range(2):
                nc.vector.tensor_copy(sin_sb[:, half * SC:(half + 1) * SC],
                                      trig_ps[(0, half)][:])
                nc.scalar.activation(cos_sb[:, half * SC:(half + 1) * SC],
                                     trig_ps[(1, half)][:],
                                     mybir.ActivationFunctionType.Copy)

        _WBASE = {0: 0, 1: D, 2: A_END, 3: A_END + D}  # wq, wv, wo, mask

        def wslice(idx, cc, col0, ncol):
            base = _WBASE[idx]
            return w_sb[cc][:, base + col0: base + col0 + ncol]

        def bias_ap(idx, cc):
            return w_sb[cc][:, 2 * D + idx: 2 * D + idx + 1]

        def act2(tag, width=S, dtype=BF16):
            return [apool.tile([128, width], dtype, tag=f"{tag}{cc}",
                               name=f"{tag}{cc}") for cc in range(2)]

        # f32 bias copies for the DVE-side evicts (GpSimd cannot touch PSUM)
        bq32 = [cpool.tile([128, 1], F32, tag=f"bq32_{c}", name=f"bq32_{c}")
                for c in range(2)]
        bo32 = cpool.tile([128, 1], F32, tag="bo32_1", name="bo32_1")
        for c in range(2):
            nc.vector.tensor_copy(bq32[c][:], bias_ap(0, c))
        nc.vector.tensor_copy(bo32[:], bias_ap(2, 1))

        qT = act2("qT")
        vT = act2("vT")
        rqT = act2("rqT")
        rvT = act2("rvT")
        # natural-layout tiles padded to 272-col st-blocks: breaks the 4KB
        # power-of-2 row-stride SBUF bank pattern that slows gram LDWEIGHTS
        DP = D + 16
        rq_nat = apool.tile([128, 8 * DP], BF16, tag="rq_nat")
        rv_nat = apool.tile([128, 8 * DP], BF16, tag="rv_nat")
        rq_nat3 = rq_nat[:].rearrange("p (st c) -> p st c", c=DP)
        rv_nat3 = rv_nat[:].rearrange("p (st c) -> p st c", c=DP)

        # ---- projections: tT[a, s] = sum_d w[d, a] x[d, s] ----
        ps_map = {}

        def proj_chunk(widx, sc):
            for ac in range(2):
                ps = pp.tile([128, SC], F32, tag="mm", bufs=5,
                             name=f"proj_ps{widx}{ac}{sc}")
                for dc in range(2):
                    nc.tensor.matmul(
                        ps[:],
                        wslice(widx, dc, ac * 128, 128),
                        xT_sb[dc][:, sc * SC:(sc + 1) * SC],
                        start=(dc == 0), stop=(dc == 1),
                    )
                ps_map[(widx, ac, sc)] = ps

        proj_chunk(0, 0)
        proj_chunk(0, 1)
        proj_chunk(1, 0)
        proj_chunk(1, 1)

        def evict(widx, dstT, ac, sc):
            src = ps_map[(widx, ac, sc)]
            dst = dstT[ac][:, sc * SC:(sc + 1) * SC]
            nc.scalar.activation(dst, src[:],
                                 mybir.ActivationFunctionType.Identity,
                                 bias=bias_ap(widx, ac))

        # rope product+combine for one (tensor, s-chunk) on DVE; the
        # per-piece half-width transposes fire right after real/imag land
        def rope_chunk(srcT, dstT, sc, mtag, natT3, tengs):
            sl = slice(sc * SC, (sc + 1) * SC)
            E, O = srcT[0][:, sl], srcT[1][:, sl]
            ssl = sin_sb[:, sl]
            csl = cos_sb[:, sl]
            m1 = opool.tile([128, SC], BF16, tag=mtag + "1", bufs=2)
            m2 = opool.tile([128, SC], BF16, tag=mtag + "2", bufs=2)
            m3 = opool.tile([128, SC], BF16, tag=mtag + "3", bufs=2)
            m4 = opool.tile([128, SC], BF16, tag=mtag + "4", bufs=2)
            # E-products first (frees the E psum bank early), then O
            nc.vector.tensor_tensor(m1[:], E, ssl, mybir.AluOpType.mult)
            nc.vector.tensor_tensor(m3[:], E, csl, mybir.AluOpType.mult)
            nc.vector.tensor_tensor(m2[:], O, csl, mybir.AluOpType.mult)
            nc.vector.tensor_tensor(m4[:], O, ssl, mybir.AluOpType.mult)
            nc.vector.tensor_tensor(dstT[0][:, sl], m1[:], m2[:],
                                    mybir.AluOpType.subtract)
            tengs[0].dma_start(natT3[:, 4 * sc:4 * sc + 4, 0:128],
                               dstT[0][:, sl], transpose=True)
            nc.vector.tensor_tensor(dstT[1][:, sl], m3[:], m4[:],
                                    mybir.AluOpType.add)
            tengs[1].dma_start(natT3[:, 4 * sc:4 * sc + 4, 128:256],
                               dstT[1][:, sl], transpose=True)

        # evicts on Scalar (PSUM-side; they overlap the input tail and the
        # DVE rope); rope all on DVE (concurrent Pool work poisons DVE);
        # evict order tracks rope consumption: q fully first, then v
        evict(0, qT, 0, 0)
        evict(0, qT, 1, 0)
        evict(0, qT, 0, 1)
        evict(0, qT, 1, 1)
        rope_chunk(qT, rqT, 0, "mq", rq_nat3, (nc.sync, nc.sync))
        evict(1, vT, 0, 0)
        evict(1, vT, 1, 0)
        rope_chunk(qT, rqT, 1, "mq", rq_nat3, (nc.sync, nc.scalar))
        evict(1, vT, 0, 1)
        evict(1, vT, 1, 1)
        rope_chunk(vT, rvT, 0, "mv", rv_nat3, (nc.sync, nc.scalar))
        rope_chunk(vT, rvT, 1, "mv", rv_nat3, (nc.sync, nc.scalar))

        # ---- Gram: Hm[e, f] = sum_s rv[s, e] rq[s, f]; mask/4 -> BDT ----
        bdt = act2("bdt", width=D)
        for ec in range(2):
            ps = pp.tile([128, SC], F32, tag="mm", bufs=5, name=f"gram_ps{ec}")
            for st in range(8):
                nc.tensor.matmul(
                    ps[:, 0:D],
                    rv_nat[:, st * DP + ec * 128: st * DP + (ec + 1) * 128],
                    rq_nat[:, st * DP: st * DP + D],
                    start=(st == 0), stop=(st == 7),
                )
            nc.vector.tensor_tensor(
                bdt[ec][:], ps[:, 0:D], wslice(3, ec, 0, D),
                mybir.AluOpType.mult)

        # ---- W2[f, o] = sum_e BDT[e, f] wot[e, o] (mask carried the 1/4) ----
        w2 = act2("w2", width=D)
        for fc in range(2):
            ps = pp.tile([128, SC], F32, tag="mm", bufs=5, name=f"w2_ps{fc}")
            for ec in range(2):
                nc.tensor.matmul(
                    ps[:, 0:D],
                    bdt[ec][:, fc * 128:(fc + 1) * 128],
                    wslice(2, ec, 0, D),
                    start=(ec == 0), stop=(ec == 1),
                )
            if fc == 0:
                nc.scalar.activation(w2[fc][:], ps[:, 0:D],
                                     mybir.ActivationFunctionType.Copy)
            else:
                nc.vector.tensor_copy(w2[fc][:], ps[:, 0:D])

        # ---- final: outT[o, s] = sum_f W2[f, o] rqT[f, s] + bo ----
        for oc in range(2):
            for sc in range(NS):
                ps = pp.tile([128, SC], F32, tag="mm", bufs=5,
                             name=f"fin_ps{oc}{sc}")
                for fc in range(2):
                    nc.tensor.matmul(
                        ps[:],
                        w2[fc][:, oc * 128:(oc + 1) * 128],
                        rqT[fc][:, sc * SC:(sc + 1) * SC],
                        start=(fc == 0), stop=(fc == 1),
                    )
                ot = opool.tile([128, SC], BF16, tag="out_sb", name=f"out_sb{oc}{sc}")
                if oc == 0:
                    nc.scalar.activation(ot[:], ps[:],
                                         mybir.ActivationFunctionType.Identity,
                                         bias=bias_ap(2, oc))
                else:
                    nc.vector.tensor_scalar(ot[:], ps[:], bo32[:], None,
                                            mybir.AluOpType.add)
                eng = nc.scalar if (oc + sc) % 2 == 0 else nc.sync
                eng.dma_start(
                    outT[oc * 128:(oc + 1) * 128, sc * SC:(sc + 1) * SC], ot[:])


_NC_CACHE = None


def _get_nc():
    global _NC_CACHE
    if _NC_CACHE is None:
        _NC_CACHE = build_kernel()
    return _NC_CACHE


def make_in_maps(x, wq_w, wq_b, wv_w, wv_b, wo_w, wo_b):
    trig, mask = _host_tables()
    wq_p = np.ascontiguousarray(wq_w[PERM].T).astype(ml_dtypes.bfloat16)
    wv_p = np.ascontiguousarray(wv_w[PERM].T).astype(ml_dtypes.bfloat16)
    wo_p = np.ascontiguousarray(wo_w[:, PERM].T).astype(ml_dtypes.bfloat16)
    bias3 = np.stack([wq_b[PERM], wv_b[PERM], wo_b], axis=1).astype(ml_dtypes.bfloat16)
    wbig = np.ascontiguousarray(
        np.concatenate([wq_p, wv_p, bias3,
                        np.zeros((256, 1), dtype=ml_dtypes.bfloat16),
                        wo_p, mask], axis=1))
    aux = np.ascontiguousarray(trig)
    in_maps = []
    for b in range(B):
        in_maps.append({
            "xT": np.ascontiguousarray(x[b].T).astype(ml_dtypes.bfloat16),
            "wbig": wbig,
            "aux": aux,
        })
    return in_maps


TRACE = False
RUN_KWARGS = {}
LAST_RESULT = None


def kernel(x, wq_w, wq_b, wk_w, wk_b, wv_w, wv_b, wo_w, wo_b):
    global LAST_RESULT
    x = np.asarray(x, dtype=np.float32)
    in_maps = make_in_maps(x, np.asarray(wq_w, np.float32), np.asarray(wq_b, np.float32),
                           np.asarray(wv_w, np.float32), np.asarray(wv_b, np.float32),
                           np.asarray(wo_w, np.float32), np.asarray(wo_b, np.float32))
    nc = _get_nc()
    res = run_bass_kernel_spmd(nc, in_maps, core_ids=list(range(N_CORES)),
                               trace=TRACE, **RUN_KWARGS)
    LAST_RESULT = res
    outs = [np.ascontiguousarray(res.results[b]["outT"].T) for b in range(B)]
    return np.stack(outs).astype(np.float32)


# revision 34
# speedup vs baseline: 1.0435x; 1.0435x over previous
"""Trainium2 Bass kernel for nn_Attention_77824807403911 (sparse_attention).

Math (per batch element, no softmax => associativity):
    q = x @ Wq^T + bq ; v = x @ Wv^T + bv          [1024, 256]
    rq = rope(q) ; rv = rope(v)
    per head h (16 heads, hd=16):  att_h = rq_h @ (rq_h^T @ rv_h) / 4
    out = att @ Wo^T + bo

Instead of the 1024x1024 score matrix we compute the 16x16 Gram per head
(64x fewer flops), realized as a full 256x256 Gram masked to the
block-diagonal, folded with Wo into a single per-batch [256,256] weight:
    F[e,f]  = sum_s rv[s,e] rq[s,f]       (Gram, transposed blocks)
    BDT     = F .* (blockmask/4)
    W2[f,o] = sum_e BDT[e,f] * Wo[o,e]
    outT    = W2^T @ rqT + bo             ([256, 1024])

Sharding: data-parallel over batch, 1 element per core, no collectives.

v12 deltas over v6:
- trig tables are no longer DMAed as [256,1024] (512KB): only 2 unique
  rows exist per table (theta in {1, 1e-4}), so a [4,1152] aux tensor
  (rows + 0/1 indicator columns) is row-broadcast on the PE via two-
  partition indicator matmuls, saving ~500KB of input traffic;
- mask carries the 1/sqrt(hd)=1/4 scale (host-folded), W2 evict is a
  plain copy;
- three DMA queues: HW rings carry wq|wv + most of x; the Pool SWDGE
  queue (4KB packets, otherwise-idle engine) carries the trig rows, one
  x piece and the late-needed wo|mask block;
- rope stays DVE-only and evicts Scalar-only: concurrent GpSimd
  elementwise slows DVE ~3x (shared SBUF read ports), while Scalar's
  PSUM-side ACTIVATE does not contend;
- rope op order is E-products first (frees each PSUM bank early) and q
  before v; per-piece half-width transposes fire right after each
  real/imag combine so the gram's first s-chunks accumulate during rope;
- PSUM: warm+trig share a 3-buffer tag, matmuls rotate through 5.
"""

import numpy as np
import ml_dtypes

import concourse.bass as bass
import concourse.bacc as bacc
import concourse.tile as tile
from concourse import mybir
from concourse.bass_utils import run_bass_kernel_spmd

B, S, D, H, HD = 8, 1024, 256, 16, 16
N_CORES = 8
BF16 = mybir.dt.bfloat16
F32 = mybir.dt.float32

PERM = np.concatenate(
    [np.arange(0, 128, 2), np.arange(128, 256, 2),
     np.arange(1, 128, 2), np.arange(129, 256, 2)]
)

WB = 4 * D + 4          # 1028 packed weight columns
A_END = 2 * D + 4       # wq|wv|bias3|pad piece


def _host_tables():
    s = np.arange(S, dtype=np.float64) + 1.0
    trig = np.stack([np.sin(s), np.sin(1e-4 * s),
                     np.cos(s), np.cos(1e-4 * s)])
    # indicator columns: row-broadcast weights (partition p < 64 -> row 0)
    p = np.arange(128)
    ind = np.stack([(p < 64), (p >= 64), (p < 64), (p >= 64)])
    aux = np.concatenate([trig, ind], axis=1).astype(ml_dtypes.bfloat16)
    a = np.arange(256)
    headp = (a % 128) // 8
    mask = 0.25 * (headp[:, None] == headp[None, :])
    return aux, mask.astype(ml_dtypes.bfloat16)


def build_kernel():
    nc = bacc.Bacc()
    xT = nc.declare_dram_parameter("xT", [D, S], BF16, isOutput=False)
    # wbig columns: [wq(256) | wv(256) | bias(3) | pad(1) | wo(256) | mask(256)]
    wbig = nc.declare_dram_parameter("wbig", [D, WB], BF16, isOutput=False)
    # aux rows: sinA, sinB, cosA, cosB (theta=1, 1e-4) + indicator cols
    aux = nc.declare_dram_parameter("aux", [4, S + 128], BF16, isOutput=False)
    outT = nc.declare_dram_parameter("outT", [D, S], BF16, isOutput=True)

    with tile.TileContext(nc) as tc:
        _body(tc, xT, wbig, aux, outT)
    nc.compile()
    return nc


def _body(tc, xT, wbig, aux, outT):
    nc = tc.nc
    NS = 2          # s chunks of 512 for matmul streaming
    SC = S // NS    # 512

    with (
        tc.tile_pool(name="const", bufs=1) as cpool,
        tc.tile_pool(name="acts", bufs=1) as apool,
        tc.tile_pool(name="psum", bufs=4, space="PSUM") as pp,
        tc.tile_pool(name="outp", bufs=4) as opool,
    ):
        # PE warm-up: garbage matmuls release the HAM clock gate while the
        # real inputs stream in.
        scratch = cpool.tile([128, 512], BF16, tag="scratch", name="scratch")
        nc.gpsimd.memset(scratch[:], 0.25)
        warm_ps = pp.tile([128, 512], F32, tag="warm", bufs=3, name="warm_ps")
        for wi in range(4):
            nc.tensor.matmul(warm_ps[:], scratch[:, 0:128], scratch[:],
                             start=True, stop=True, skip_group_check=True)

        xT_sb, w_sb = [], []
        for cc in range(2):
            xT_sb.append(cpool.tile([128, S], BF16, tag=f"xT{cc}", name=f"xT{cc}"))
            w_sb.append(cpool.tile([128, WB], BF16, tag=f"wbig{cc}",
                                   name=f"wbig{cc}"))
        sin_sb = cpool.tile([128, S], BF16, tag="sin", name="sin_sb")
        cos_sb = cpool.tile([128, S], BF16, tag="cos", name="cos_sb")
        sinrows = cpool.tile([2, S + 128], BF16, tag="sinrows", name="sinrows")
        cosrows = cpool.tile([2, S + 128], BF16, tag="cosrows", name="cosrows")

        # ---- input DMAs: HW rings carry only wq|wv + x (the critical
        # path); tiny aux rows + late-needed wo|mask ride the Pool SWDGE
        # queue (4KB packets, otherwise-idle engine) ----
        # three queues: HW rings carry wq + most of x; the Pool SWDGE
        # queue carries the trig rows, one x piece, and the late wo|mask
        nc.gpsimd.dma_start(xT_sb[1][:, SC:S], xT[128:256, SC:S])
        nc.sync.dma_start(w_sb[0][:, 0:D], wbig[0:128, 0:D])
        nc.scalar.dma_start(w_sb[1][:, 0:D], wbig[128:256, 0:D])
        # tiny trig rows as ring piece #2: the queue is already ramped, and
        # they land ~1.5us earlier than on the SWDGE queue
        nc.sync.dma_start(sinrows[:], aux[0:2, :])
        nc.scalar.dma_start(cosrows[:], aux[2:4, :])
        nc.sync.dma_start(xT_sb[0][:, 0:SC], xT[0:128, 0:SC])
        nc.scalar.dma_start(xT_sb[1][:, 0:SC], xT[128:256, 0:SC])
        nc.sync.dma_start(xT_sb[0][:, SC:S], xT[0:128, SC:S])
        nc.scalar.dma_start(w_sb[1][:, D:A_END], wbig[128:256, D:A_END])
        nc.sync.dma_start(w_sb[0][:, D:A_END], wbig[0:128, D:A_END])
        nc.gpsimd.dma_start(w_sb[0][:, A_END:WB], wbig[0:128, A_END:WB])
        nc.gpsimd.dma_start(w_sb[1][:, A_END:WB], wbig[128:256, A_END:WB])

        # ---- trig tables: PE row-broadcast (only 2 unique rows/table);
        # pinned to the front of the PE stream so the monotonic engine
        # counters don't chain them behind x-gated projection matmuls ----
        trig_ps = {}
        with tc.high_priority():
            # s0 halves of both tables first (earliest rope consumers)
            for t, half, rows in ((0, 0, sinrows), (1, 0, cosrows),
                                  (0, 1, sinrows), (1, 1, cosrows)):
                ps = pp.tile([128, SC], F32, tag="warm", bufs=3,
                             name=f"trig_ps{t}{half}")
                nc.tensor.matmul(
                    ps[:],
                    rows[0:2, S:S + 128],
                    rows[0:2, half * SC:(half + 1) * SC],
                    start=True, stop=True,
                )
                trig_ps[(t, half)] = ps
            # sin evicts in the DVE bubble, cos evicts in the Scalar bubble
            for half in range(2):
                nc.vector.tensor_copy(sin_sb[:, half * SC:(half + 1) * SC],
                                      trig_ps[(0, half)][:])
                nc.scalar.activation(cos_sb[:, half * SC:(half + 1) * SC],
                                     trig_ps[(1, half)][:],
                                     mybir.ActivationFunctionType.Copy)

        _WBASE = {0: 0, 1: D, 2: A_END, 3: A_END + D}  # wq, wv, wo, mask

        def wslice(idx, cc, col0, ncol):
            base = _WBASE[idx]
            return w_sb[cc][:, base + col0: base + col0 + ncol]

        def bias_ap(idx, cc):
            return w_sb[cc][:, 2 * D + idx: 2 * D + idx + 1]

        def act2(tag, width=S, dtype=BF16):
            return [apool.tile([128, width], dtype, tag=f"{tag}{cc}",
                               name=f"{tag}{cc}") for cc in range(2)]

        # f32 copy of bo for the DVE-side final evicts; emitted later so it
        # doesn't head-of-line-block the DVE behind the late bias DMA piece
        bo32 = cpool.tile([128, 1], F32, tag="bo32_1", name="bo32_1")

        qT = act2("qT")
        vT = act2("vT")
        rqT = act2("rqT")
        rvT = act2("rvT")
        # natural-layout tiles padded to 272-col st-blocks: breaks the 4KB
        # power-of-2 row-stride SBUF bank pattern that slows gram LDWEIGHTS
        DP = D + 16
        rq_nat = apool.tile([128, 8 * DP], BF16, tag="rq_nat")
        rv_nat = apool.tile([128, 8 * DP], BF16, tag="rv_nat")
        rq_nat3 = rq_nat[:].rearrange("p (st c) -> p st c", c=DP)
        rv_nat3 = rv_nat[:].rearrange("p (st c) -> p st c", c=DP)

        # ---- projections: tT[a, s] = sum_d w[d, a] x[d, s] ----
        ps_map = {}

        def proj_chunk(widx, sc):
            for ac in range(2):
                ps = pp.tile([128, SC], F32, tag="mm", bufs=5,
                             name=f"proj_ps{widx}{ac}{sc}")
                for dc in range(2):
                    nc.tensor.matmul(
                        ps[:],
                        wslice(widx, dc, ac * 128, 128),
                        xT_sb[dc][:, sc * SC:(sc + 1) * SC],
                        start=(dc == 0), stop=(dc == 1),
                    )
                ps_map[(widx, ac, sc)] = ps

        proj_chunk(0, 0)
        proj_chunk(0, 1)
        proj_chunk(1, 0)
        proj_chunk(1, 1)

        def evict(widx, dstT, ac, sc):
            src = ps_map[(widx, ac, sc)]
            dst = dstT[ac][:, sc * SC:(sc + 1) * SC]
            nc.scalar.activation(dst, src[:],
                                 mybir.ActivationFunctionType.Identity,
                                 bias=bias_ap(widx, ac))

        # rope product+combine for one (tensor, s-chunk) on DVE; the
        # per-piece half-width transposes fire right after real/imag land
        def rope_chunk(srcT, dstT, sc, mtag, natT3, tengs):
            sl = slice(sc * SC, (sc + 1) * SC)
            E, O = srcT[0][:, sl], srcT[1][:, sl]
            ssl = sin_sb[:, sl]
            csl = cos_sb[:, sl]
            m1 = opool.tile([128, SC], BF16, tag=mtag + "1", bufs=2)
            m2 = opool.tile([128, SC], BF16, tag=mtag + "2", bufs=2)
            m3 = opool.tile([128, SC], BF16, tag=mtag + "3", bufs=2)
            m4 = opool.tile([128, SC], BF16, tag=mtag + "4", bufs=2)
            # E-products first (frees the E psum bank early), then O
            nc.vector.tensor_tensor(m1[:], E, ssl, mybir.AluOpType.mult)
            nc.vector.tensor_tensor(m3[:], E, csl, mybir.AluOpType.mult)
            nc.vector.tensor_tensor(m2[:], O, csl, mybir.AluOpType.mult)
            nc.vector.tensor_tensor(m4[:], O, ssl, mybir.AluOpType.mult)
            nc.vector.tensor_tensor(dstT[0][:, sl], m1[:], m2[:],
                                    mybir.AluOpType.subtract)
            tengs[0].dma_start(natT3[:, 4 * sc:4 * sc + 4, 0:128],
                               dstT[0][:, sl], transpose=True)
            nc.vector.tensor_tensor(dstT[1][:, sl], m3[:], m4[:],
                                    mybir.AluOpType.add)
            tengs[1].dma_start(natT3[:, 4 * sc:4 * sc + 4, 128:256],
                               dstT[1][:, sl], transpose=True)

        # evicts on Scalar (PSUM-side; they overlap the input tail and the
        # DVE rope); rope all on DVE (concurrent Pool work poisons DVE);
        # evict order tracks rope consumption: q fully first, then v
        evict(0, qT, 0, 0)
        evict(0, qT, 1, 0)
        evict(0, qT, 0, 1)
        evict(0, qT, 1, 1)
        rope_chunk(qT, rqT, 0, "mq", rq_nat3, (nc.sync, nc.sync))
        evict(1, vT, 0, 0)
        evict(1, vT, 1, 0)
        rope_chunk(qT, rqT, 1, "mq", rq_nat3, (nc.sync, nc.scalar))
        evict(1, vT, 0, 1)
        evict(1, vT, 1, 1)
        rope_chunk(vT, rvT, 0, "mv", rv_nat3, (nc.sync, nc.scalar))
        rope_chunk(vT, rvT, 1, "mv", rv_nat3, (nc.sync, nc.scalar))
        nc.vector.tensor_copy(bo32[:], bias_ap(2, 1))

        # ---- Gram: Hm[e, f] = sum_s rv[s, e] rq[s, f]; mask/4 -> BDT ----
        bdt = act2("bdt", width=D)
        for ec in range(2):
            ps = pp.tile([128, SC], F32, tag="mm", bufs=5, name=f"gram_ps{ec}")
            for st in range(8):
                nc.tensor.matmul(
                    ps[:, 0:D],
                    rv_nat[:, st * DP + ec * 128: st * DP + (ec + 1) * 128],
                    rq_nat[:, st * DP: st * DP + D],
                    start=(st == 0), stop=(st == 7),
                )
            nc.vector.tensor_tensor(
                bdt[ec][:], ps[:, 0:D], wslice(3, ec, 0, D),
                mybir.AluOpType.mult)

        # ---- W2[f, o] = sum_e BDT[e, f] wot[e, o] (mask carried the 1/4) ----
        w2 = act2("w2", width=D)
        for fc in range(2):
            ps = pp.tile([128, SC], F32, tag="mm", bufs=5, name=f"w2_ps{fc}")
            for ec in range(2):
                nc.tensor.matmul(
                    ps[:, 0:D],
                    bdt[ec][:, fc * 128:(fc + 1) * 128],
                    wslice(2, ec, 0, D),
                    start=(ec == 0), stop=(ec == 1),
                )
            if fc == 0:
                nc.scalar.activation(w2[fc][:], ps[:, 0:D],
                                     mybir.ActivationFunctionType.Copy)
            else:
                nc.vector.tensor_copy(w2[fc][:], ps[:, 0:D])

        # ---- final: outT[o, s] = sum_f W2[f, o] rqT[f, s] + bo ----
        for oc in range(2):
            for sc in range(NS):
                ps = pp.tile([128, SC], F32, tag="mm", bufs=5,
                             name=f"fin_ps{oc}{sc}")
                for fc in range(2):
                    nc.tensor.matmul(
                        ps[:],
                        w2[fc][:, oc * 128:(oc + 1) * 128],
                        rqT[fc][:, sc * SC:(sc + 1) * SC],
                        start=(fc == 0), stop=(fc == 1),
                    )
                ot = opool.tile([128, SC], BF16, tag="out_sb", name=f"out_sb{oc}{sc}")
                if oc == 0:
                    nc.scalar.activation(ot[:], ps[:],
                                         mybir.ActivationFunctionType.Identity,
                                         bias=bias_ap(2, oc))
                else:
                    nc.vector.tensor_scalar(ot[:], ps[:], bo32[:], None,
                                            mybir.AluOpType.add)
                eng = nc.scalar if (oc + sc) % 2 == 0 else nc.sync
                eng.dma_start(
                    outT[oc * 128:(oc + 1) * 128, sc * SC:(sc + 1) * SC], ot[:])


_NC_CACHE = None


def _get_nc():
    global _NC_CACHE
    if _NC_CACHE is None:
        _NC_CACHE = build_kernel()
    return _NC_CACHE


def make_in_maps(x, wq_w, wq_b, wv_w, wv_b, wo_w, wo_b):
    trig, mask = _host_tables()
    wq_p = np.ascontiguousarray(wq_w[PERM].T).astype(ml_dtypes.bfloat16)
    wv_p = np.ascontiguousarray(wv_w[PERM].T).astype(ml_dtypes.bfloat16)
    wo_p = np.ascontiguousarray(wo_w[:, PERM].T).astype(ml_dtypes.bfloat16)
    bias3 = np.stack([wq_b[PERM], wv_b[PERM], wo_b], axis=1).astype(ml_dtypes.bfloat16)
    wbig = np.ascontiguousarray(
        np.concatenate([wq_p, wv_p, bias3,
                        np.zeros((256, 1), dtype=ml_dtypes.bfloat16),
                        wo_p, mask], axis=1))
    aux = np.ascontiguousarray(trig)
    in_maps = []
    for b in range(B):
        in_maps.append({
            "xT": np.ascontiguousarray(x[b].T).astype(ml_dtypes.bfloat16),
            "wbig": wbig,
            "aux": aux,
        })
    return in_maps


TRACE = False
RUN_KWARGS = {}
LAST_RESULT = None


def kernel(x, wq_w, wq_b, wk_w, wk_b, wv_w, wv_b, wo_w, wo_b):
    global LAST_RESULT
    x = np.asarray(x, dtype=np.float32)
    in_maps = make_in_maps(x, np.asarray(wq_w, np.float32), np.asarray(wq_b, np.float32),
                           np.asarray(wv_w, np.float32), np.asarray(wv_b, np.float32),
                           np.asarray(wo_w, np.float32), np.asarray(wo_b, np.float32))
    nc = _get_nc()
    res = run_bass_kernel_spmd(nc, in_maps, core_ids=list(range(N_CORES)),
                               trace=TRACE, **RUN_KWARGS)
    LAST_RESULT = res
    outs = [np.ascontiguousarray(res.results[b]["outT"].T) for b in range(B)]
    return np.stack(outs).astype(np.float32)


# revision 35
# speedup vs baseline: 1.0508x; 1.0070x over previous
"""Trainium2 Bass kernel for nn_Attention_77824807403911 (sparse_attention).

Math (per batch element, no softmax => associativity):
    q = x @ Wq^T + bq ; v = x @ Wv^T + bv          [1024, 256]
    rq = rope(q) ; rv = rope(v)
    per head h (16 heads, hd=16):  att_h = rq_h @ (rq_h^T @ rv_h) / 4
    out = att @ Wo^T + bo

Instead of the 1024x1024 score matrix we compute the 16x16 Gram per head
(64x fewer flops), realized as a full 256x256 Gram masked to the
block-diagonal, folded with Wo into a single per-batch [256,256] weight:
    F[e,f]  = sum_s rv[s,e] rq[s,f]       (Gram, transposed blocks)
    BDT     = F .* (blockmask/4)
    W2[f,o] = sum_e BDT[e,f] * Wo[o,e]
    outT    = W2^T @ rqT + bo             ([256, 1024])

Sharding: data-parallel over batch, 1 element per core, no collectives.

v12 deltas over v6:
- trig tables are no longer DMAed as [256,1024] (512KB): only 2 unique
  rows exist per table (theta in {1, 1e-4}), so a [4,1152] aux tensor
  (rows + 0/1 indicator columns) is row-broadcast on the PE via two-
  partition indicator matmuls, saving ~500KB of input traffic;
- mask carries the 1/sqrt(hd)=1/4 scale (host-folded), W2 evict is a
  plain copy;
- three DMA queues: HW rings carry wq|wv + most of x; the Pool SWDGE
  queue (4KB packets, otherwise-idle engine) carries the trig rows, one
  x piece and the late-needed wo|mask block;
- rope stays DVE-only and evicts Scalar-only: concurrent GpSimd
  elementwise slows DVE ~3x (shared SBUF read ports), while Scalar's
  PSUM-side ACTIVATE does not contend;
- rope op order is E-products first (frees each PSUM bank early) and q
  before v; per-piece half-width transposes fire right after each
  real/imag combine so the gram's first s-chunks accumulate during rope;
- PSUM: warm+trig share a 3-buffer tag, matmuls rotate through 5.
"""

import numpy as np
import ml_dtypes

import concourse.bass as bass
import concourse.bacc as bacc
import concourse.tile as tile
from concourse import mybir
from concourse.bass_utils import run_bass_kernel_spmd

B, S, D, H, HD = 8, 1024, 256, 16, 16
N_CORES = 8
BF16 = mybir.dt.bfloat16
F32 = mybir.dt.float32

PERM = np.concatenate(
    [np.arange(0, 128, 2), np.arange(128, 256, 2),
     np.arange(1, 128, 2), np.arange(129, 256, 2)]
)

WB = 4 * D + 4          # 1028 packed weight columns
A_END = 2 * D + 4       # wq|wv|bias3|pad piece


def _host_tables():
    s = np.arange(S, dtype=np.float64) + 1.0
    trig = np.stack([np.sin(s), np.sin(1e-4 * s),
                     np.cos(s), np.cos(1e-4 * s)])
    # indicator columns: row-broadcast weights (partition p < 64 -> row 0)
    p = np.arange(128)
    ind = np.stack([(p < 64), (p >= 64), (p < 64), (p >= 64)])
    aux = np.concatenate([trig, ind], axis=1).astype(ml_dtypes.bfloat16)
    a = np.arange(256)
    headp = (a % 128) // 8
    mask = 0.25 * (headp[:, None] == headp[None, :])
    return aux, mask.astype(ml_dtypes.bfloat16)


def build_kernel():
    nc = bacc.Bacc()
    xT = nc.declare_dram_parameter("xT", [D, S], BF16, isOutput=False)
    # wbig columns: [wq(256) | wv(256) | bias(3) | pad(1) | wo(256) | mask(256)]
    wbig = nc.declare_dram_parameter("wbig", [D, WB], BF16, isOutput=False)
    # aux rows: sinA, sinB, cosA, cosB (theta=1, 1e-4) + indicator cols
    aux = nc.declare_dram_parameter("aux", [4, S + 128], BF16, isOutput=False)
    outT = nc.declare_dram_parameter("outT", [D, S], BF16, isOutput=True)

    with tile.TileContext(nc) as tc:
        _body(tc, xT, wbig, aux, outT)
    nc.compile()
    return nc


def _body(tc, xT, wbig, aux, outT):
    nc = tc.nc
    NS = 2          # s chunks of 512 for matmul streaming
    SC = S // NS    # 512

    with (
        tc.tile_pool(name="const", bufs=1) as cpool,
        tc.tile_pool(name="acts", bufs=1) as apool,
        tc.tile_pool(name="psum", bufs=4, space="PSUM") as pp,
        tc.tile_pool(name="outp", bufs=4) as opool,
    ):
        # PE warm-up: garbage matmuls release the HAM clock gate while the
        # real inputs stream in.
        scratch = cpool.tile([128, 512], BF16, tag="scratch", name="scratch")
        nc.gpsimd.memset(scratch[:], 0.25)
        warm_ps = pp.tile([128, 512], F32, tag="warm", bufs=3, name="warm_ps")
        for wi in range(4):
            nc.tensor.matmul(warm_ps[:], scratch[:, 0:128], scratch[:],
                             start=True, stop=True, skip_group_check=True)

        xT_sb, w_sb = [], []
        for cc in range(2):
            xT_sb.append(cpool.tile([128, S], BF16, tag=f"xT{cc}", name=f"xT{cc}"))
            w_sb.append(cpool.tile([128, WB], BF16, tag=f"wbig{cc}",
                                   name=f"wbig{cc}"))
        sin_sb = cpool.tile([128, S], BF16, tag="sin", name="sin_sb")
        cos_sb = cpool.tile([128, S], BF16, tag="cos", name="cos_sb")
        sinrows = cpool.tile([2, S + 128], BF16, tag="sinrows", name="sinrows")
        cosrows = cpool.tile([2, S + 128], BF16, tag="cosrows", name="cosrows")

        # ---- input DMAs: HW rings carry only wq|wv + x (the critical
        # path); tiny aux rows + late-needed wo|mask ride the Pool SWDGE
        # queue (4KB packets, otherwise-idle engine) ----
        # three queues: HW rings carry wq + most of x; the Pool SWDGE
        # queue carries the trig rows, one x piece, and the late wo|mask
        nc.gpsimd.dma_start(sinrows[:], aux[0:2, :])
        nc.gpsimd.dma_start(cosrows[:], aux[2:4, :])
        nc.sync.dma_start(w_sb[0][:, 0:D], wbig[0:128, 0:D])
        nc.scalar.dma_start(w_sb[1][:, 0:D], wbig[128:256, 0:D])
        nc.sync.dma_start(xT_sb[0][:, 0:SC], xT[0:128, 0:SC])
        nc.scalar.dma_start(xT_sb[1][:, 0:SC], xT[128:256, 0:SC])
        nc.gpsimd.dma_start(xT_sb[1][:, SC:S], xT[128:256, SC:S])
        nc.sync.dma_start(xT_sb[0][:, SC:S], xT[0:128, SC:S])
        nc.scalar.dma_start(w_sb[1][:, D:A_END], wbig[128:256, D:A_END])
        nc.sync.dma_start(w_sb[0][:, D:A_END], wbig[0:128, D:A_END])
        nc.gpsimd.dma_start(w_sb[0][:, A_END:WB], wbig[0:128, A_END:WB])
        nc.gpsimd.dma_start(w_sb[1][:, A_END:WB], wbig[128:256, A_END:WB])

        # ---- trig tables: PE row-broadcast (only 2 unique rows/table);
        # pinned to the front of the PE stream so the monotonic engine
        # counters don't chain them behind x-gated projection matmuls ----
        trig_ps = {}
        with tc.high_priority():
            # s0 halves of both tables first (earliest rope consumers)
            for t, half, rows in ((0, 0, sinrows), (1, 0, cosrows),
                                  (0, 1, sinrows), (1, 1, cosrows)):
                ps = pp.tile([128, SC], F32, tag="warm", bufs=3,
                             name=f"trig_ps{t}{half}")
                nc.tensor.matmul(
                    ps[:],
                    rows[0:2, S:S + 128],
                    rows[0:2, half * SC:(half + 1) * SC],
                    start=True, stop=True,
                )
                trig_ps[(t, half)] = ps
            # sin evicts in the DVE bubble, cos evicts in the Scalar bubble
            for half in range(2):
                nc.vector.tensor_copy(sin_sb[:, half * SC:(half + 1) * SC],
                                      trig_ps[(0, half)][:])
                nc.scalar.activation(cos_sb[:, half * SC:(half + 1) * SC],
                                     trig_ps[(1, half)][:],
                                     mybir.ActivationFunctionType.Copy)

        _WBASE = {0: 0, 1: D, 2: A_END, 3: A_END + D}  # wq, wv, wo, mask

        def wslice(idx, cc, col0, ncol):
            base = _WBASE[idx]
            return w_sb[cc][:, base + col0: base + col0 + ncol]

        def bias_ap(idx, cc):
            return w_sb[cc][:, 2 * D + idx: 2 * D + idx + 1]

        def act2(tag, width=S, dtype=BF16):
            return [apool.tile([128, width], dtype, tag=f"{tag}{cc}",
                               name=f"{tag}{cc}") for cc in range(2)]

        # f32 copy of bo for the DVE-side final evicts; emitted later so it
        # doesn't head-of-line-block the DVE behind the late bias DMA piece
        bo32 = cpool.tile([128, 1], F32, tag="bo32_1", name="bo32_1")

        qT = act2("qT")
        vT = act2("vT")
        rqT = act2("rqT")
        rvT = act2("rvT")
        # natural-layout tiles padded to 272-col st-blocks: breaks the 4KB
        # power-of-2 row-stride SBUF bank pattern that slows gram LDWEIGHTS
        DP = D + 16
        rq_nat = apool.tile([128, 8 * DP], BF16, tag="rq_nat")
        rv_nat = apool.tile([128, 8 * DP], BF16, tag="rv_nat")
        rq_nat3 = rq_nat[:].rearrange("p (st c) -> p st c", c=DP)
        rv_nat3 = rv_nat[:].rearrange("p (st c) -> p st c", c=DP)

        # ---- projections: tT[a, s] = sum_d w[d, a] x[d, s] ----
        ps_map = {}

        def proj_chunk(widx, sc):
            for ac in range(2):
                ps = pp.tile([128, SC], F32, tag="mm", bufs=5,
                             name=f"proj_ps{widx}{ac}{sc}")
                for dc in range(2):
                    nc.tensor.matmul(
                        ps[:],
                        wslice(widx, dc, ac * 128, 128),
                        xT_sb[dc][:, sc * SC:(sc + 1) * SC],
                        start=(dc == 0), stop=(dc == 1),
                    )
                ps_map[(widx, ac, sc)] = ps

        proj_chunk(0, 0)
        proj_chunk(0, 1)
        proj_chunk(1, 0)
        proj_chunk(1, 1)

        def evict(widx, dstT, ac, sc):
            src = ps_map[(widx, ac, sc)]
            dst = dstT[ac][:, sc * SC:(sc + 1) * SC]
            nc.scalar.activation(dst, src[:],
                                 mybir.ActivationFunctionType.Identity,
                                 bias=bias_ap(widx, ac))

        # rope product+combine for one (tensor, s-chunk) on DVE; the
        # per-piece half-width transposes fire right after real/imag land
        def rope_chunk(srcT, dstT, sc, mtag, natT3, tengs):
            sl = slice(sc * SC, (sc + 1) * SC)
            E, O = srcT[0][:, sl], srcT[1][:, sl]
            ssl = sin_sb[:, sl]
            csl = cos_sb[:, sl]
            m1 = opool.tile([128, SC], BF16, tag=mtag + "1", bufs=2)
            m2 = opool.tile([128, SC], BF16, tag=mtag + "2", bufs=2)
            m3 = opool.tile([128, SC], BF16, tag=mtag + "3", bufs=2)
            m4 = opool.tile([128, SC], BF16, tag=mtag + "4", bufs=2)
            # E-products first (frees the E psum bank early), then O
            nc.vector.tensor_tensor(m1[:], E, ssl, mybir.AluOpType.mult)
            nc.vector.tensor_tensor(m3[:], E, csl, mybir.AluOpType.mult)
            nc.vector.tensor_tensor(m2[:], O, csl, mybir.AluOpType.mult)
            nc.vector.tensor_tensor(m4[:], O, ssl, mybir.AluOpType.mult)
            nc.vector.tensor_tensor(dstT[0][:, sl], m1[:], m2[:],
                                    mybir.AluOpType.subtract)
            tengs[0].dma_start(natT3[:, 4 * sc:4 * sc + 4, 0:128],
                               dstT[0][:, sl], transpose=True)
            nc.vector.tensor_tensor(dstT[1][:, sl], m3[:], m4[:],
                                    mybir.AluOpType.add)
            tengs[1].dma_start(natT3[:, 4 * sc:4 * sc + 4, 128:256],
                               dstT[1][:, sl], transpose=True)

        # evicts on Scalar (PSUM-side; they overlap the input tail and the
        # DVE rope); rope all on DVE (concurrent Pool work poisons DVE);
        # evict order tracks rope consumption: q fully first, then v
        evict(0, qT, 0, 0)
        evict(0, qT, 1, 0)
        evict(0, qT, 0, 1)
        evict(0, qT, 1, 1)
        rope_chunk(qT, rqT, 0, "mq", rq_nat3, (nc.sync, nc.sync))
        evict(1, vT, 0, 0)
        evict(1, vT, 1, 0)
        rope_chunk(qT, rqT, 1, "mq", rq_nat3, (nc.sync, nc.scalar))
        evict(1, vT, 0, 1)
        evict(1, vT, 1, 1)
        rope_chunk(vT, rvT, 0, "mv", rv_nat3, (nc.sync, nc.scalar))
        rope_chunk(vT, rvT, 1, "mv", rv_nat3, (nc.sync, nc.scalar))
        nc.vector.tensor_copy(bo32[:], bias_ap(2, 1))

        # ---- Gram: Hm[e, f] = sum_s rv[s, e] rq[s, f]; mask/4 -> BDT ----
        bdt = act2("bdt", width=D)
        for ec in range(2):
            ps = pp.tile([128, SC], F32, tag="mm", bufs=5, name=f"gram_ps{ec}")
            for st in range(8):
                nc.tensor.matmul(
                    ps[:, 0:D],
                    rv_nat[:, st * DP + ec * 128: st * DP + (ec + 1) * 128],
                    rq_nat[:, st * DP: st * DP + D],
                    start=(st == 0), stop=(st == 7),
                )
            nc.vector.tensor_tensor(
                bdt[ec][:], ps[:, 0:D], wslice(3, ec, 0, D),
                mybir.AluOpType.mult)

        # ---- W2[f, o] = sum_e BDT[e, f] wot[e, o] (mask carried the 1/4) ----
        w2 = act2("w2", width=D)
        for fc in range(2):
            ps = pp.tile([128, SC], F32, tag="mm", bufs=5, name=f"w2_ps{fc}")
            for ec in range(2):
                nc.tensor.matmul(
                    ps[:, 0:D],
                    bdt[ec][:, fc * 128:(fc + 1) * 128],
                    wslice(2, ec, 0, D),
                    start=(ec == 0), stop=(ec == 1),
                )
            if fc == 0:
                nc.scalar.activation(w2[fc][:], ps[:, 0:D],
                                     mybir.ActivationFunctionType.Copy)
            else:
                nc.vector.tensor_copy(w2[fc][:], ps[:, 0:D])

        # ---- final: outT[o, s] = sum_f W2[f, o] rqT[f, s] + bo ----
        for oc in range(2):
            for sc in range(NS):
                ps = pp.tile([128, SC], F32, tag="mm", bufs=5,
                             name=f"fin_ps{oc}{sc}")
                for fc in range(2):
                    nc.tensor.matmul(
                        ps[:],
                        w2[fc][:, oc * 128:(oc + 1) * 128],
                        rqT[fc][:, sc * SC:(sc + 1) * SC],
                        start=(fc == 0), stop=(fc == 1),
                    )
                ot = opool.tile([128, SC], BF16, tag="out_sb", name=f"out_sb{oc}{sc}")
                if oc == 0:
                    nc.scalar.activation(ot[:], ps[:],
                                         mybir.ActivationFunctionType.Identity,
                                         bias=bias_ap(2, oc))
                else:
                    nc.vector.tensor_scalar(ot[:], ps[:], bo32[:], None,
                                            mybir.AluOpType.add)
                eng = nc.scalar if (oc + sc) % 2 == 0 else nc.sync
                eng.dma_start(
                    outT[oc * 128:(oc + 1) * 128, sc * SC:(sc + 1) * SC], ot[:])


_NC_CACHE = None


def _get_nc():
    global _NC_CACHE
    if _NC_CACHE is None:
        _NC_CACHE = build_kernel()
    return _NC_CACHE


def make_in_maps(x, wq_w, wq_b, wv_w, wv_b, wo_w, wo_b):
    trig, mask = _host_tables()
    wq_p = np.ascontiguousarray(wq_w[PERM].T).astype(ml_dtypes.bfloat16)
    wv_p = np.ascontiguousarray(wv_w[PERM].T).astype(ml_dtypes.bfloat16)
    wo_p = np.ascontiguousarray(wo_w[:, PERM].T).astype(ml_dtypes.bfloat16)
    bias3 = np.stack([wq_b[PERM], wv_b[PERM], wo_b], axis=1).astype(ml_dtypes.bfloat16)
    wbig = np.ascontiguousarray(
        np.concatenate([wq_p, wv_p, bias3,
                        np.zeros((256, 1), dtype=ml_dtypes.bfloat16),
                        wo_p, mask], axis=1))
    aux = np.ascontiguousarray(trig)
    in_maps = []
    for b in range(B):
        in_maps.append({
            "xT": np.ascontiguousarray(x[b].T).astype(ml_dtypes.bfloat16),
            "wbig": wbig,
            "aux": aux,
        })
    return in_maps


TRACE = False
RUN_KWARGS = {}
LAST_RESULT = None


def kernel(x, wq_w, wq_b, wk_w, wk_b, wv_w, wv_b, wo_w, wo_b):
    global LAST_RESULT
    x = np.asarray(x, dtype=np.float32)
    in_maps = make_in_maps(x, np.asarray(wq_w, np.float32), np.asarray(wq_b, np.float32),
                           np.asarray(wv_w, np.float32), np.asarray(wv_b, np.float32),
                           np.asarray(wo_w, np.float32), np.asarray(wo_b, np.float32))
    nc = _get_nc()
    res = run_bass_kernel_spmd(nc, in_maps, core_ids=list(range(N_CORES)),
                               trace=TRACE, **RUN_KWARGS)
    LAST_RESULT = res
    outs = [np.ascontiguousarray(res.results[b]["outT"].T) for b in range(B)]
    return np.stack(outs).astype(np.float32)
